# revision 1
# baseline (speedup 1.0000x reference)
"""FAVOR+ (Performer) causal linear attention with rotary embeddings on 8 TRN2 cores.

Reference computation (B=2, L=4096, H=8, D=64, M=256):
  q,k <- GPT-J rotary(q, k, sinu_pos)
  qp = relu(rot_q @ projT / sqrt(M)) + EPS   [B,L,H,M]
  kp = relu(rot_k @ projT / sqrt(M)) + EPS
  causal scan over L: KV_l = sum_{j<=l} kp_j (x) [v_j, 1];  out_l = (qp_l @ KV_l)[:D] / (qp_l @ KV_l)[D]

Sharding: 16 (b,h) pairs, 2 per core (pure data parallel, no collectives).
Per core, per pair: chunked scan with C=128 chunk, KV state [M, D+1] held in
PSUM (matmul accumulation across chunks):
  per chunk: rotary (DVE/GPSIMD) -> PE transpose -> feature matmuls ->
  AT = kp qp^T (masked causal) -> num = qp@KV_prev + maskedAT^T@[v,1] -> out
  KV += kp^T @ [v,1]
"""

import sys
import os

for _p in ("/opt/trn_rl_repo", "/root/.axon_site/_ro/trn_rl_repo"):
    if os.path.isdir(_p) and _p not in sys.path:
        sys.path.insert(0, _p)

import numpy as np
import concourse.bass as bass
import concourse.mybir as mybir
import concourse.tile as tile
from concourse.bass_utils import run_bass_kernel_spmd
from concourse.masks import make_identity

B, L, H, D, M = 2, 4096, 8, 64, 256
EPS = 1e-3
C = 128                 # chunk length
NCH = L // C            # 32 chunks
NCORES = 8
PAIRS_PER_CORE = (B * H) // NCORES  # 2
F32 = mybir.dt.float32


def _legalize_sync_waits(nc):
    """This image's walrus supports ONE sync-wait slot per instruction.
    Split multi-wait instructions into preceding single-wait EventSemaphore
    ops on the same engine (same-engine execution is in-order, so
    sequential waits == AND of waits)."""
    for f in nc.m.functions:
        for b in f.blocks:
            insts = b.instructions
            new = []
            dirty = False
            for ins in insts:
                si = ins.sync_info
                if si is not None and si.on_wait is not None and len(si.on_wait) > 1:
                    waits = list(si.on_wait)
                    for j, wt in enumerate(waits[:-1]):
                        es = mybir.InstEventSemaphore(
                            name=f"{ins.name}_xw{j}",
                            engine=ins.engine,
                            ins=[],
                            outs=[],
                            sync_info=mybir.SyncInfo(on_wait=[wt], on_update=[]),
                        )
                        new.append(es)
                    ins.sync_info = mybir.SyncInfo(
                        on_wait=[waits[-1]], on_update=list(si.on_update or [])
                    )
                    dirty = True
                if si is not None and si.on_update is not None and len(si.on_update) > 1:
                    raise AssertionError(
                        f"multi-update on {ins.name} ({ins.opcode}) unsupported"
                    )
                new.append(ins)
            if dirty:
                b.instructions = new


def _build_program():
    nc = bass.Bass()

    qk_in = []   # per pair: (q, k, v) dram handles
    outs = []
    for p in range(PAIRS_PER_CORE):
        qd = nc.dram_tensor(f"q{p}", [L, D], F32, kind="ExternalInput")
        kd = nc.dram_tensor(f"k{p}", [L, D], F32, kind="ExternalInput")
        vd = nc.dram_tensor(f"v{p}", [L, D], F32, kind="ExternalInput")
        qk_in.append((qd, kd, vd))
        outs.append(nc.dram_tensor(f"o{p}", [L, D], F32, kind="ExternalOutput"))
    cos2_d = nc.dram_tensor("cos2", [L, 2 * D], F32, kind="ExternalInput")
    sin2_d = nc.dram_tensor("sin2", [L, 2 * D], F32, kind="ExternalInput")
    projt_d = nc.dram_tensor("projt", [D, M], F32, kind="ExternalInput")
    mask_d = nc.dram_tensor("maskat", [C, C], F32, kind="ExternalInput")

    with tile.TileContext(nc) as tc:
        with (
            tc.tile_pool(name="consts", bufs=1) as consts,
            tc.tile_pool(name="stream", bufs=3) as stream,
            tc.tile_pool(name="feat", bufs=2) as feat,
            tc.tile_pool(name="kvp", bufs=2) as kvpool,
            tc.tile_pool(name="psA", bufs=2, space="PSUM") as psA,
            tc.tile_pool(name="psB", bufs=1, space="PSUM") as psB,
            tc.tile_pool(name="pskv", bufs=1, space="PSUM") as pskv,
        ):
            # ---- constants in SBUF ----
            cos_sb = consts.tile([128, NCH, 2 * D], F32)
            sin_sb = consts.tile([128, NCH, 2 * D], F32)
            nc.sync.dma_start(cos_sb[:], cos2_d.rearrange("(c p) j -> p c j", p=128))
            nc.sync.dma_start(sin_sb[:], sin2_d.rearrange("(c p) j -> p c j", p=128))
            projt = consts.tile([D, M], F32)
            nc.sync.dma_start(projt[:], projt_d[:])
            maskat = consts.tile([C, C], F32)
            nc.sync.dma_start(maskat[:], mask_d[:])
            ident = consts.tile([128, 128], F32)
            make_identity(nc, ident[:])

            for p in range(PAIRS_PER_CORE):
                qd, kd, vd = qk_in[p]
                od = outs[p]
                # persistent KV state for this pair: [m_sub][128, D+1]
                kv_ps = [
                    pskv.tile([128, D + 1], F32, name=f"kv{p}_{m}", tag=f"kv{m}")
                    for m in range(2)
                ]
                for ci in range(NCH):
                    lo = ci * C
                    # -------- load --------
                    xqk = stream.tile([128, 128], F32, tag="xqk")
                    nc.sync.dma_start(xqk[:, 0:D], qd[lo : lo + C, :])
                    nc.sync.dma_start(xqk[:, D : 2 * D], kd[lo : lo + C, :])
                    v_aug = stream.tile([128, D + 1], F32, tag="vaug")
                    nc.sync.dma_start(v_aug[:, 0:D], vd[lo : lo + C, :])
                    nc.gpsimd.memset(v_aug[:, D : D + 1], 1.0)

                    # -------- rotary: rot = x*cos2 + swap(x)*sin2alt --------
                    cslice = cos_sb[:, ci, :]
                    sslice = sin_sb[:, ci, :]
                    x_sw = xqk.rearrange("p (t two) -> p t two", two=2)[:, :, ::-1]
                    t2 = stream.tile([128, 128], F32, tag="t2")
                    nc.gpsimd.tensor_tensor(
                        t2[:].rearrange("p (t two) -> p t two", two=2),
                        x_sw,
                        sslice.rearrange("p (t two) -> p t two", two=2),
                        mybir.AluOpType.mult,
                    )
                    rot = stream.tile([128, 128], F32, tag="rot")
                    nc.vector.tensor_tensor(rot[:], xqk[:], cslice, mybir.AluOpType.mult)
                    nc.vector.tensor_tensor(rot[:], rot[:], t2[:], mybir.AluOpType.add)

                    # -------- transpose to [D, 2C]: qT | kT --------
                    ps_t = psB.tile([D, 2 * C], F32, tag="pt")
                    nc.tensor.transpose(ps_t[:, 0:C], rot[:, 0:D], ident[:])
                    nc.tensor.transpose(ps_t[:, C : 2 * C], rot[:, D : 2 * D], ident[:])
                    rotT = feat.tile([D, 2 * C], F32, tag="rotT")
                    nc.scalar.copy(rotT[:], ps_t[:])

                    # -------- features --------
                    # fsb layout: [:, 0:128]=qpT_m0, [:,128:256]=kpT_m0,
                    #             [:, 256:384]=qpT_m1, [:,384:512]=kpT_m1
                    ps_f = psA.tile([128, 2 * 2 * C], F32, tag="pf")
                    nc.tensor.matmul(
                        ps_f[:, 0 : 2 * C], projt[:, 0:128], rotT[:],
                        start=True, stop=True,
                    )
                    nc.tensor.matmul(
                        ps_f[:, 2 * C : 4 * C], projt[:, 128:256], rotT[:],
                        start=True, stop=True,
                    )
                    fsb = feat.tile([128, 2 * 2 * C], F32, tag="fsb")
                    nc.vector.tensor_scalar(
                        fsb[:], ps_f[:], 0.0, EPS,
                        mybir.AluOpType.max, mybir.AluOpType.add,
                    )
                    qpT = [fsb[:, 0:C], fsb[:, 2 * C : 3 * C]]
                    kpT = [fsb[:, C : 2 * C], fsb[:, 3 * C : 4 * C]]

                    if ci < NCH - 1:
                        # kp in [C, M] layout (lhsT for the KV update)
                        ps_kp = psB.tile([C, M], F32, tag="pk")
                        nc.tensor.matmul(
                            ps_kp[:], rotT[:, C : 2 * C], projt[:],
                            start=True, stop=True,
                        )
                        kp_sb = feat.tile([C, M], F32, tag="kpsb")
                        nc.vector.tensor_scalar(
                            kp_sb[:], ps_kp[:], 0.0, EPS,
                            mybir.AluOpType.max, mybir.AluOpType.add,
                        )

                    # -------- AT = kp qp^T (this chunk), causal-masked --------
                    ps_a = psB.tile([C, C], F32, tag="pa")
                    nc.tensor.matmul(ps_a[:], kpT[0], qpT[0], start=True, stop=False)
                    nc.tensor.matmul(ps_a[:], kpT[1], qpT[1], start=False, stop=True)
                    at_sb = feat.tile([C, C], F32, tag="atsb")
                    nc.vector.tensor_tensor(
                        at_sb[:], ps_a[:], maskat[:], mybir.AluOpType.mult
                    )

                    # -------- snapshot KV state (chunks < ci) --------
                    if ci > 0:
                        kv_sb = kvpool.tile([128, 2, D + 1], F32, tag="kvsb")
                        nc.scalar.copy(kv_sb[:, 0, :], kv_ps[0][:])
                        nc.scalar.copy(kv_sb[:, 1, :], kv_ps[1][:])

                    # -------- numerator/denominator --------
                    ps_o = psB.tile([C, D + 1], F32, tag="po")
                    if ci > 0:
                        nc.tensor.matmul(
                            ps_o[:], qpT[0], kv_sb[:, 0, :], start=True, stop=False
                        )
                        nc.tensor.matmul(
                            ps_o[:], qpT[1], kv_sb[:, 1, :], start=False, stop=False
                        )
                        nc.tensor.matmul(
                            ps_o[:], at_sb[:], v_aug[:], start=False, stop=True
                        )
                    else:
                        nc.tensor.matmul(
                            ps_o[:], at_sb[:], v_aug[:], start=True, stop=True
                        )

                    # -------- KV += kp^T v_aug --------
                    if ci < NCH - 1:
                        for m in range(2):
                            nc.tensor.matmul(
                                kv_ps[m][:],
                                kp_sb[:, m * 128 : (m + 1) * 128],
                                v_aug[:],
                                start=(ci == 0),
                                stop=(ci == NCH - 2),
                            )

                    # -------- out = num / den --------
                    rec = feat.tile([C, 1], F32, tag="rec")
                    nc.vector.reciprocal(rec[:], ps_o[:, D : D + 1])
                    osb = feat.tile([C, D], F32, tag="osb")
                    nc.vector.tensor_tensor(
                        osb[:], ps_o[:, 0:D], rec[:].to_broadcast([C, D]),
                        mybir.AluOpType.mult,
                    )
                    nc.sync.dma_start(od[lo : lo + C, :], osb[:])

    _legalize_sync_waits(nc)
    return nc


_PROGRAM_CACHE = {}


def _get_program():
    if "nc" not in _PROGRAM_CACHE:
        _PROGRAM_CACHE["nc"] = _build_program()
    return _PROGRAM_CACHE["nc"]


def kernel(q, k, v, sinu_pos, proj):
    q = np.asarray(q, np.float32)
    k = np.asarray(k, np.float32)
    v = np.asarray(v, np.float32)
    sinu = np.asarray(sinu_pos, np.float32)[0]          # [L, D]
    proj = np.asarray(proj, np.float32)                 # [M, D]

    half = D // 2
    sin = sinu[:, :half]
    cos = sinu[:, half:]
    sin_i = np.repeat(sin, 2, axis=-1)                  # [L, D]
    cos_i = np.repeat(cos, 2, axis=-1)
    sinalt = sin_i.copy()
    sinalt[:, 0::2] *= -1.0
    cos2 = np.ascontiguousarray(np.concatenate([cos_i, cos_i], axis=1))   # [L, 2D]
    sin2 = np.ascontiguousarray(np.concatenate([sinalt, sinalt], axis=1))
    projt = np.ascontiguousarray(proj.T / np.sqrt(np.float32(M))).astype(np.float32)
    maskat = np.triu(np.ones((C, C), np.float32))       # [c2, c1] = c2 <= c1

    pairs = [(b, h) for b in range(B) for h in range(H)]
    nc = _get_program()
    in_maps = []
    for core in range(NCORES):
        im = {
            "cos2": cos2,
            "sin2": sin2,
            "projt": projt,
            "maskat": maskat,
        }
        for p in range(PAIRS_PER_CORE):
            b, h = pairs[core * PAIRS_PER_CORE + p]
            im[f"q{p}"] = np.ascontiguousarray(q[b, :, h, :])
            im[f"k{p}"] = np.ascontiguousarray(k[b, :, h, :])
            im[f"v{p}"] = np.ascontiguousarray(v[b, :, h, :])
        in_maps.append(im)

    res = run_bass_kernel_spmd(nc, in_maps, core_ids=list(range(NCORES)))

    out = np.empty((B, L, H, D), np.float32)
    for core in range(NCORES):
        for p in range(PAIRS_PER_CORE):
            b, h = pairs[core * PAIRS_PER_CORE + p]
            out[b, :, h, :] = res.results[core][f"o{p}"]
    return out


# revision 11
# speedup vs baseline: 1.0237x; 1.0237x over previous
"""FAVOR+ (Performer) causal linear attention with rotary embeddings on 8 TRN2 cores.

Reference computation (B=2, L=4096, H=8, D=64, M=256):
  q,k <- GPT-J rotary(q, k, sinu_pos)
  qp = relu(rot_q @ projT / sqrt(M)) + EPS   [B,L,H,M]
  kp = relu(rot_k @ projT / sqrt(M)) + EPS
  causal scan over L: KV_l = sum_{j<=l} kp_j (x) [v_j, 1];  out_l = (qp_l @ KV_l)[:D] / (qp_l @ KV_l)[D]

Sharding: 16 (b,h) pairs, 2 per core (pure data parallel, no collectives).
Per core, per pair: chunked scan with C=128 chunks; the KV state [M, D+1]
lives in PSUM and accumulates across chunks via matmul accumulation.

Numerics: matmul operands are bf16 (PSUM accumulation in fp32); rotary math
and the final num/den division are fp32. fp32 matmuls on TRN2 are emulated
as 2 bf16 passes with doubled weight loads, so bf16 operands halve PE time.

Per chunk: rotary (DVE/GPSIMD, fp32 in / bf16 out) -> XBAR DMA transpose
(SBUF->SBUF, off the PE) -> feature matmuls with q on PE rows 0:63 and k on
rows 64:127 (concurrent row groups) -> AT = kp qp^T causal-masked ->
num = qp@KV_prev + maskedAT^T@[v,1] -> out = num/den; KV += kp^T@[v,1].
"""

import sys
import os

for _p in ("/opt/trn_rl_repo", "/root/.axon_site/_ro/trn_rl_repo"):
    if os.path.isdir(_p) and _p not in sys.path:
        sys.path.insert(0, _p)

import numpy as np
import ml_dtypes
import concourse.bass as bass
import concourse.mybir as mybir
import concourse.tile as tile
from concourse.bass_utils import run_bass_kernel_spmd

B, L, H, D, M = 2, 4096, 8, 64, 256
EPS = 1e-3
C = 128                 # chunk length
NCH = L // C            # 32 chunks
NCORES = 8
PAIRS_PER_CORE = (B * H) // NCORES  # 2
F32 = mybir.dt.float32
BF16 = mybir.dt.bfloat16
USE_XBAR = os.environ.get("K_USE_XBAR", "1") == "1"


def _legalize_sync_waits(nc):
    """This image's walrus supports ONE sync-wait slot per instruction.
    Split multi-wait instructions into preceding single-wait EventSemaphore
    ops on the same engine (same-engine execution is in-order, so
    sequential waits == AND of waits)."""
    for f in nc.m.functions:
        for b in f.blocks:
            insts = b.instructions
            new = []
            dirty = False
            for ins in insts:
                si = ins.sync_info
                if si is not None and si.on_wait is not None and len(si.on_wait) > 1:
                    waits = list(si.on_wait)
                    for j, wt in enumerate(waits[:-1]):
                        es = mybir.InstEventSemaphore(
                            name=f"{ins.name}_xw{j}",
                            engine=ins.engine,
                            ins=[],
                            outs=[],
                            sync_info=mybir.SyncInfo(on_wait=[wt], on_update=[]),
                        )
                        new.append(es)
                    ins.sync_info = mybir.SyncInfo(
                        on_wait=[waits[-1]], on_update=list(si.on_update or [])
                    )
                    dirty = True
                if si is not None and si.on_update is not None and len(si.on_update) > 1:
                    raise AssertionError(
                        f"multi-update on {ins.name} ({ins.opcode}) unsupported"
                    )
                new.append(ins)
            if dirty:
                b.instructions = new


def _build_program(legalize=True):
    nc = bass.Bass()

    qk_in = []
    outs = []
    for p in range(PAIRS_PER_CORE):
        qd = nc.dram_tensor(f"q{p}", [L, D], F32, kind="ExternalInput")
        kd = nc.dram_tensor(f"k{p}", [L, D], F32, kind="ExternalInput")
        vd = nc.dram_tensor(f"v{p}", [L, D + 1], BF16, kind="ExternalInput")
        qk_in.append((qd, kd, vd))
        outs.append(nc.dram_tensor(f"o{p}", [L, D], F32, kind="ExternalOutput"))
    cos2_d = nc.dram_tensor("cos2", [L, 2 * D], F32, kind="ExternalInput")
    sin2_d = nc.dram_tensor("sin2", [L, 2 * D], F32, kind="ExternalInput")
    projt_d = nc.dram_tensor("projt", [D, M], BF16, kind="ExternalInput")
    mask_d = nc.dram_tensor("maskat", [C, C], F32, kind="ExternalInput")

    with tile.TileContext(nc) as tc:
        with (
            tc.tile_pool(name="consts", bufs=1) as consts,
            tc.tile_pool(name="stream", bufs=3) as stream,
            tc.tile_pool(name="feat", bufs=2) as feat,
            tc.tile_pool(name="kvp", bufs=2) as kvpool,
            tc.tile_pool(name="psA", bufs=2, space="PSUM") as psA,
            tc.tile_pool(name="psO", bufs=2 if USE_XBAR else 1, space="PSUM") as psO,
            tc.tile_pool(name="pskv", bufs=1, space="PSUM") as pskv,
        ):
            # ---- constants in SBUF ----
            cos_sb = consts.tile([128, NCH, 2 * D], F32)
            sin_sb = consts.tile([128, NCH, 2 * D], F32)
            nc.sync.dma_start(cos_sb[:], cos2_d.rearrange("(c p) j -> p c j", p=128))
            nc.sync.dma_start(sin_sb[:], sin2_d.rearrange("(c p) j -> p c j", p=128))
            # projt duplicated on partitions 0:64 (q row-group) and 64:128 (k)
            projt2 = consts.tile([128, M], BF16)
            nc.sync.dma_start(projt2[0:D, :], projt_d[:])
            nc.sync.dma_start(projt2[D : 2 * D, :], projt_d[:])
            maskat = consts.tile([C, C], F32)
            nc.sync.dma_start(maskat[:], mask_d[:])
            if not USE_XBAR:
                from concourse.masks import make_identity

                ident = consts.tile([128, 128], BF16)
                make_identity(nc, ident[:])

            for p in range(PAIRS_PER_CORE):
                qd, kd, vd = qk_in[p]
                od = outs[p]
                kv_ps = [
                    pskv.tile([128, D + 1], F32, name=f"kv{p}_{m}", tag=f"kv{m}")
                    for m in range(2)
                ]
                for ci in range(NCH):
                    lo = ci * C
                    # -------- load --------
                    xqk = stream.tile([128, 128], F32, tag="xqk")
                    nc.sync.dma_start(xqk[:, 0:D], qd[lo : lo + C, :])
                    nc.sync.dma_start(xqk[:, D : 2 * D], kd[lo : lo + C, :])
                    v_aug = stream.tile([128, D + 1], BF16, tag="vaug")
                    nc.sync.dma_start(v_aug[:], vd[lo : lo + C, :])

                    # -------- rotary: rot = x*cos2 + swap(x)*sin2alt (bf16 out) ----
                    cslice = cos_sb[:, ci, :]
                    sslice = sin_sb[:, ci, :]
                    x_sw = xqk.rearrange("p (t two) -> p t two", two=2)[:, :, ::-1]
                    t2 = stream.tile([128, 128], F32, tag="t2")
                    nc.gpsimd.tensor_tensor(
                        t2[:].rearrange("p (t two) -> p t two", two=2),
                        x_sw,
                        sslice.rearrange("p (t two) -> p t two", two=2),
                        mybir.AluOpType.mult,
                    )
                    t1 = stream.tile([128, 128], F32, tag="t1")
                    nc.vector.tensor_tensor(t1[:], xqk[:], cslice, mybir.AluOpType.mult)
                    rot = stream.tile([128, 128], BF16, tag="rot")
                    nc.vector.tensor_tensor(rot[:], t1[:], t2[:], mybir.AluOpType.add)

                    # -------- transpose: rotT[j, l] = rot[l, j] --------
                    # rows 0:63 = q dims, 64:127 = k dims
                    rotT = feat.tile([128, C], BF16, tag="rotT")
                    if USE_XBAR:
                        nc.sync.dma_start_transpose(rotT[:], rot[:])
                    else:
                        ps_t = psA.tile([128, C], BF16, tag="pt")
                        nc.tensor.transpose(ps_t[:], rot[:], ident[:])
                        nc.vector.tensor_copy(rotT[:], ps_t[:])

                    # -------- features (q on PE rows 0:63, k on rows 64:127) ------
                    # Concurrent row-group matmuls MUST drain into different
                    # PSUM banks (same-bank concurrent drains crash the HW):
                    # q-features -> ps_fq bank; k-features + kp -> ps_fk bank.
                    ps_fq = psA.tile([128, 256], F32, tag="pfq")
                    ps_fk = psA.tile([128, 512], F32, tag="pfk")
                    for m in range(2):
                        nc.tensor.matmul(
                            ps_fq[:, m * 128 : (m + 1) * 128],
                            projt2[0:D, m * 128 : (m + 1) * 128],
                            rotT[0:D, :],
                            start=True, stop=True,
                        )
                        nc.tensor.matmul(
                            ps_fk[:, m * 128 : (m + 1) * 128],
                            projt2[D : 2 * D, m * 128 : (m + 1) * 128],
                            rotT[D : 2 * D, :],
                            start=True, stop=True,
                        )
                    fsb = feat.tile([128, 512], BF16, tag="fsb")
                    nc.vector.tensor_scalar(
                        fsb[:, 0:256], ps_fq[:], 0.0, EPS,
                        mybir.AluOpType.max, mybir.AluOpType.add,
                    )
                    nc.vector.tensor_scalar(
                        fsb[:, 256:512], ps_fk[:, 0:256], 0.0, EPS,
                        mybir.AluOpType.max, mybir.AluOpType.add,
                    )
                    qpT = [fsb[:, 0:128], fsb[:, 128:256]]
                    kpT = [fsb[:, 256:384], fsb[:, 384:512]]

                    if ci < NCH - 1:
                        # kp in [C, M] layout (lhsT of the KV update), k row-group
                        nc.tensor.matmul(
                            ps_fk[:, 256:512], rotT[D : 2 * D, :],
                            projt2[D : 2 * D, :],
                            start=True, stop=True,
                        )
                        kp_sb = feat.tile([C, M], BF16, tag="kpsb")
                        nc.vector.tensor_scalar(
                            kp_sb[:], ps_fk[:, 256:512], 0.0, EPS,
                            mybir.AluOpType.max, mybir.AluOpType.add,
                        )

                    # -------- AT = kp qp^T (this chunk), causal-masked --------
                    # shares the ps_o bank (cols 128:256); all K=128 matmuls
                    # use the full row range so they serialize on the PE.
                    ps_oa = psO.tile([C, 256], F32, tag="po")
                    ps_a = ps_oa[:, 128:256]
                    nc.tensor.matmul(ps_a, kpT[0], qpT[0], start=True, stop=False)
                    nc.tensor.matmul(ps_a, kpT[1], qpT[1], start=False, stop=True)
                    at_sb = feat.tile([C, C], BF16, tag="atsb")
                    nc.vector.tensor_tensor(
                        at_sb[:], ps_a, maskat[:], mybir.AluOpType.mult
                    )

                    # -------- snapshot KV state (chunks < ci) --------
                    if ci > 0:
                        kv_sb = kvpool.tile([128, 2, D + 2], BF16, tag="kvsb")
                        nc.scalar.copy(kv_sb[:, 0, 0 : D + 1], kv_ps[0][:])
                        nc.scalar.copy(kv_sb[:, 1, 0 : D + 1], kv_ps[1][:])

                    # -------- numerator/denominator --------
                    ps_o = ps_oa[:, 0 : D + 1]
                    if ci > 0:
                        nc.tensor.matmul(
                            ps_o, qpT[0], kv_sb[:, 0, 0 : D + 1], start=True, stop=False
                        )
                        nc.tensor.matmul(
                            ps_o, qpT[1], kv_sb[:, 1, 0 : D + 1], start=False, stop=False
                        )
                        nc.tensor.matmul(
                            ps_o, at_sb[:], v_aug[:], start=False, stop=True
                        )
                    else:
                        nc.tensor.matmul(
                            ps_o, at_sb[:], v_aug[:], start=True, stop=True
                        )

                    # -------- KV += kp^T v_aug --------
                    if ci < NCH - 1:
                        for m in range(2):
                            nc.tensor.matmul(
                                kv_ps[m][:],
                                kp_sb[:, m * 128 : (m + 1) * 128],
                                v_aug[:],
                                start=(ci == 0),
                                stop=True,
                                skip_group_check=True,
                            )

                    # -------- out = num / den --------
                    rec = feat.tile([C, 1], F32, tag="rec")
                    nc.vector.reciprocal(rec[:], ps_oa[:, D : D + 1])
                    osb = feat.tile([C, D], F32, tag="osb")
                    nc.vector.tensor_tensor(
                        osb[:], ps_oa[:, 0:D], rec[:].to_broadcast([C, D]),
                        mybir.AluOpType.mult,
                    )
                    nc.sync.dma_start(od[lo : lo + C, :], osb[:])

    if legalize:
        _legalize_sync_waits(nc)
    return nc


_PROGRAM_CACHE = {}


def _get_program():
    if "nc" not in _PROGRAM_CACHE:
        _PROGRAM_CACHE["nc"] = _build_program()
    return _PROGRAM_CACHE["nc"]


def kernel(q, k, v, sinu_pos, proj):
    q = np.asarray(q, np.float32)
    k = np.asarray(k, np.float32)
    v = np.asarray(v, np.float32)
    sinu = np.asarray(sinu_pos, np.float32)[0]          # [L, D]
    proj = np.asarray(proj, np.float32)                 # [M, D]

    half = D // 2
    sin = sinu[:, :half]
    cos = sinu[:, half:]
    sin_i = np.repeat(sin, 2, axis=-1)                  # [L, D]
    cos_i = np.repeat(cos, 2, axis=-1)
    sinalt = sin_i.copy()
    sinalt[:, 0::2] *= -1.0
    cos2 = np.ascontiguousarray(np.concatenate([cos_i, cos_i], axis=1))   # [L, 2D]
    sin2 = np.ascontiguousarray(np.concatenate([sinalt, sinalt], axis=1))
    projt = np.ascontiguousarray(proj.T / np.sqrt(np.float32(M))).astype(
        ml_dtypes.bfloat16
    )
    maskat = np.triu(np.ones((C, C), np.float32))       # [c2, c1] = c2 <= c1
    ones_col = np.ones((L, 1), np.float32)

    pairs = [(b, h) for b in range(B) for h in range(H)]
    nc = _get_program()
    in_maps = []
    for core in range(NCORES):
        im = {
            "cos2": cos2,
            "sin2": sin2,
            "projt": projt,
            "maskat": maskat,
        }
        for p in range(PAIRS_PER_CORE):
            b, h = pairs[core * PAIRS_PER_CORE + p]
            im[f"q{p}"] = np.ascontiguousarray(q[b, :, h, :])
            im[f"k{p}"] = np.ascontiguousarray(k[b, :, h, :])
            im[f"v{p}"] = np.ascontiguousarray(
                np.concatenate([v[b, :, h, :], ones_col], axis=1)
            ).astype(ml_dtypes.bfloat16)
        in_maps.append(im)

    res = run_bass_kernel_spmd(nc, in_maps, core_ids=list(range(NCORES)))

    out = np.empty((B, L, H, D), np.float32)
    for core in range(NCORES):
        for p in range(PAIRS_PER_CORE):
            b, h = pairs[core * PAIRS_PER_CORE + p]
            out[b, :, h, :] = res.results[core][f"o{p}"]
    return out


# revision 12
# speedup vs baseline: 2.1246x; 2.0754x over previous
"""FAVOR+ (Performer) causal linear attention with rotary embeddings on 8 TRN2 cores.

Reference computation (B=2, L=4096, H=8, D=64, M=256):
  q,k <- GPT-J rotary(q, k, sinu_pos)
  qp = relu(rot_q @ projT / sqrt(M)) + EPS   [B,L,H,M]
  kp = relu(rot_k @ projT / sqrt(M)) + EPS
  causal scan over L: KV_l = sum_{j<=l} kp_j (x) [v_j, 1];  out_l = (qp_l @ KV_l)[:D] / (qp_l @ KV_l)[D]

Sharding: 16 (b,h) pairs, 2 per core (pure data parallel, no collectives).
Per core: chunked scan with C=128 chunks; the two pairs are interleaved
chunk-by-chunk so their independent dependency chains fill each other's
cross-engine stalls. The KV state [M, D+1] lives in PSUM (one bank per
pair, both m-halves packed) and accumulates across chunks via matmul
accumulation (has_written bits).

Numerics: all matmul operands are bf16, accumulation fp32 in PSUM; the
final num/den division is fp32. Measured end-to-end relative error vs the
fp32 reference ~2e-3 (dominated by bf16 rounding of matmul operands).

Hardware notes baked in here:
 - fp32 matmuls on TRN2 are emulated as 2 bf16 passes (2x instructions,
   2x weight loads) -> use bf16 operands.
 - Matmuls on disjoint PE row groups execute CONCURRENTLY; two such
   matmuls draining into the same PSUM bank crash the device. q-features
   (rows 0:63) and k-features (rows 64:127) therefore write separate banks.
 - This walrus build supports ONE sync-wait slot per instruction;
   _legalize_sync_waits splits multi-wait instructions.
"""

import sys
import os

for _p in ("/opt/trn_rl_repo", "/root/.axon_site/_ro/trn_rl_repo"):
    if os.path.isdir(_p) and _p not in sys.path:
        sys.path.insert(0, _p)

import numpy as np
import ml_dtypes
import concourse.bass as bass
import concourse.mybir as mybir
import concourse.tile as tile
from concourse.bass_utils import run_bass_kernel_spmd
from concourse.masks import make_identity

B, L, H, D, M = 2, 4096, 8, 64, 256
EPS = 1e-3
C = 128                 # chunk length
NCH = L // C            # 32 chunks
NCORES = 8
PAIRS_PER_CORE = (B * H) // NCORES  # 2
F32 = mybir.dt.float32
BF16 = mybir.dt.bfloat16

# kv PSUM packing: m0 at cols [0:65], m1 at cols [68:133] (16B-aligned)
KV1 = 68
KVW = 136


def _legalize_sync_waits(nc):
    """Split multi-wait instructions into preceding single-wait
    EventSemaphore ops on the same engine (same-engine execution is
    in-order, so sequential waits == AND of waits)."""
    for f in nc.m.functions:
        for b in f.blocks:
            insts = b.instructions
            new = []
            dirty = False
            for ins in insts:
                si = ins.sync_info
                if si is not None and si.on_wait is not None and len(si.on_wait) > 1:
                    waits = list(si.on_wait)
                    for j, wt in enumerate(waits[:-1]):
                        es = mybir.InstEventSemaphore(
                            name=f"{ins.name}_xw{j}",
                            engine=ins.engine,
                            ins=[],
                            outs=[],
                            sync_info=mybir.SyncInfo(on_wait=[wt], on_update=[]),
                        )
                        new.append(es)
                    ins.sync_info = mybir.SyncInfo(
                        on_wait=[waits[-1]], on_update=list(si.on_update or [])
                    )
                    dirty = True
                if si is not None and si.on_update is not None and len(si.on_update) > 1:
                    raise AssertionError(
                        f"multi-update on {ins.name} ({ins.opcode}) unsupported"
                    )
                new.append(ins)
            if dirty:
                b.instructions = new


def _build_program(legalize=True):
    nc = bass.Bass()

    qk_in = []
    outs = []
    for p in range(PAIRS_PER_CORE):
        qd = nc.dram_tensor(f"q{p}", [L, D], BF16, kind="ExternalInput")
        kd = nc.dram_tensor(f"k{p}", [L, D], BF16, kind="ExternalInput")
        vd = nc.dram_tensor(f"v{p}", [L, D + 1], BF16, kind="ExternalInput")
        qk_in.append((qd, kd, vd))
        outs.append(nc.dram_tensor(f"o{p}", [L, D], F32, kind="ExternalOutput"))
    cos2_d = nc.dram_tensor("cos2", [L, 2 * D], BF16, kind="ExternalInput")
    sin2_d = nc.dram_tensor("sin2", [L, 2 * D], BF16, kind="ExternalInput")
    projt_d = nc.dram_tensor("projt", [D, M], BF16, kind="ExternalInput")
    mask_d = nc.dram_tensor("maskat", [C, C], F32, kind="ExternalInput")

    with tile.TileContext(nc) as tc:
        with (
            tc.tile_pool(name="consts", bufs=1) as consts,
            tc.tile_pool(name="stream", bufs=4) as stream,
            tc.tile_pool(name="feat", bufs=3) as feat,
            tc.tile_pool(name="kvp", bufs=2) as kvpool,
            tc.tile_pool(name="psA", bufs=1, space="PSUM") as psA,
            tc.tile_pool(name="psT", bufs=2, space="PSUM") as psT,
            tc.tile_pool(name="psO", bufs=2, space="PSUM") as psO,
            tc.tile_pool(name="pskv", bufs=1, space="PSUM") as pskv,
        ):
            # ---- constants ----
            cos_sb = consts.tile([128, NCH, 2 * D], BF16)
            sin_sb = consts.tile([128, NCH, 2 * D], BF16)
            nc.sync.dma_start(cos_sb[:], cos2_d.rearrange("(c p) j -> p c j", p=128))
            nc.sync.dma_start(sin_sb[:], sin2_d.rearrange("(c p) j -> p c j", p=128))
            projt2 = consts.tile([128, M], BF16)
            nc.sync.dma_start(projt2[0:D, :], projt_d[:])
            nc.sync.dma_start(projt2[D : 2 * D, :], projt_d[:])
            maskat = consts.tile([C, C], F32)
            nc.sync.dma_start(maskat[:], mask_d[:])
            ident = consts.tile([128, 128], BF16)
            make_identity(nc, ident[:])

            kv_ps = [
                pskv.tile([128, KVW], F32, name=f"kvps{p}", tag=f"kv{p}")
                for p in range(PAIRS_PER_CORE)
            ]

            for ci in range(NCH):
                for p in range(PAIRS_PER_CORE):
                    qd, kd, vd = qk_in[p]
                    od = outs[p]
                    kv = kv_ps[p]
                    lo = ci * C

                    # -------- load (bf16) --------
                    xqk = stream.tile([128, 128], BF16, tag="xqk")
                    nc.sync.dma_start(xqk[:, 0:D], qd[lo : lo + C, :])
                    nc.sync.dma_start(xqk[:, D : 2 * D], kd[lo : lo + C, :])
                    v_aug = stream.tile([128, D + 1], BF16, tag="vaug")
                    nc.sync.dma_start(v_aug[:], vd[lo : lo + C, :])

                    # -------- rotary: rot = x*cos2 + swap(x)*sin2alt --------
                    cslice = cos_sb[:, ci, :]
                    sslice = sin_sb[:, ci, :]
                    x_sw = xqk.rearrange("p (t two) -> p t two", two=2)[:, :, ::-1]
                    t2 = stream.tile([128, 128], BF16, tag="t2")
                    nc.gpsimd.tensor_tensor(
                        t2[:].rearrange("p (t two) -> p t two", two=2),
                        x_sw,
                        sslice.rearrange("p (t two) -> p t two", two=2),
                        mybir.AluOpType.mult,
                    )
                    t1 = stream.tile([128, 128], BF16, tag="t1")
                    nc.vector.tensor_tensor(t1[:], xqk[:], cslice, mybir.AluOpType.mult)
                    rot = stream.tile([128, 128], BF16, tag="rot")
                    nc.gpsimd.tensor_tensor(rot[:], t1[:], t2[:], mybir.AluOpType.add)

                    # -------- PE transpose: rotT rows 0:63 = qT, 64:127 = kT ----
                    pt = psT.tile([128, 128], BF16, tag="pt")
                    nc.tensor.transpose(pt[:], rot[:], ident[:])
                    rotT = feat.tile([128, 128], BF16, tag="rotT")
                    nc.scalar.copy(rotT[:], pt[:])

                    # -------- features: q rows 0:63 / k rows 64:127 ----------
                    # concurrent row groups MUST drain to different banks
                    ps_fq = psA.tile([128, 256], F32, tag="pfq")
                    ps_fk = psA.tile([128, 512], F32, tag="pfk")
                    for m in range(2):
                        nc.tensor.matmul(
                            ps_fq[:, m * 128 : (m + 1) * 128],
                            projt2[0:D, m * 128 : (m + 1) * 128],
                            rotT[0:D, :],
                            start=True, stop=True,
                        )
                        nc.tensor.matmul(
                            ps_fk[:, m * 128 : (m + 1) * 128],
                            projt2[D : 2 * D, m * 128 : (m + 1) * 128],
                            rotT[D : 2 * D, :],
                            start=True, stop=True,
                        )
                    if ci < NCH - 1:
                        # kp[C, M] (lhsT of the KV update), k row-group
                        nc.tensor.matmul(
                            ps_fk[:, 256:512],
                            rotT[D : 2 * D, :],
                            projt2[D : 2 * D, :],
                            start=True, stop=True,
                        )

                    fsb = feat.tile([128, 512], BF16, tag="fsb")
                    nc.vector.tensor_scalar(
                        fsb[:, 0:256], ps_fq[:], 0.0, EPS,
                        mybir.AluOpType.max, mybir.AluOpType.add,
                    )
                    nc.vector.tensor_scalar(
                        fsb[:, 256:512], ps_fk[:, 0:256], 0.0, EPS,
                        mybir.AluOpType.max, mybir.AluOpType.add,
                    )
                    qpT = [fsb[:, 0:128], fsb[:, 128:256]]
                    kpT = [fsb[:, 256:384], fsb[:, 384:512]]
                    if ci < NCH - 1:
                        kp_sb = feat.tile([C, M], BF16, tag="kpsb")
                        nc.vector.tensor_scalar(
                            kp_sb[:], ps_fk[:, 256:512], 0.0, EPS,
                            mybir.AluOpType.max, mybir.AluOpType.add,
                        )

                    # -------- AT = kp qp^T, causal mask --------
                    # po bank: cols 0:65 = num/den, cols 128:256 = AT
                    po = psO.tile([C, 256], F32, tag="po")
                    ps_a = po[:, 128:256]
                    nc.tensor.matmul(ps_a, kpT[0], qpT[0], start=True, stop=False)
                    nc.tensor.matmul(ps_a, kpT[1], qpT[1], start=False, stop=True)
                    at32 = feat.tile([C, C], F32, tag="at32")
                    nc.scalar.copy(at32[:], ps_a)
                    at_sb = feat.tile([C, C], BF16, tag="atsb")
                    nc.gpsimd.tensor_tensor(
                        at_sb[:], at32[:], maskat[:], mybir.AluOpType.mult
                    )

                    # -------- snapshot KV (chunks < ci) --------
                    if ci > 0:
                        kv_sb = kvpool.tile([128, KVW], BF16, tag="kvsb")
                        nc.scalar.copy(
                            kv_sb[:, 0 : KV1 + D + 1], kv[:, 0 : KV1 + D + 1]
                        )

                    # -------- num/den --------
                    ps_o = po[:, 0 : D + 1]
                    if ci > 0:
                        nc.tensor.matmul(
                            ps_o, qpT[0], kv_sb[:, 0 : D + 1], start=True, stop=False
                        )
                        nc.tensor.matmul(
                            ps_o, qpT[1], kv_sb[:, KV1 : KV1 + D + 1],
                            start=False, stop=False,
                        )
                        nc.tensor.matmul(
                            ps_o, at_sb[:], v_aug[:], start=False, stop=True
                        )
                    else:
                        nc.tensor.matmul(
                            ps_o, at_sb[:], v_aug[:], start=True, stop=True
                        )

                    # -------- KV += kp^T v_aug (PSUM accumulate) --------
                    if ci < NCH - 1:
                        for m in range(2):
                            nc.tensor.matmul(
                                kv[:, m * KV1 : m * KV1 + D + 1],
                                kp_sb[:, m * 128 : (m + 1) * 128],
                                v_aug[:],
                                start=(ci == 0 and m == 0),
                                stop=True,
                                skip_group_check=True,
                            )

                    # -------- out = num * (1/den) --------
                    rec = feat.tile([C, 1], F32, tag="rec")
                    nc.vector.reciprocal(rec[:], po[:, D : D + 1])
                    osb = feat.tile([C, D], F32, tag="osb")
                    nc.scalar.activation(
                        osb[:], po[:, 0:D],
                        mybir.ActivationFunctionType.Copy,
                        bias=0.0, scale=rec[:],
                    )
                    nc.sync.dma_start(od[lo : lo + C, :], osb[:])

    if legalize:
        _legalize_sync_waits(nc)
    return nc


_PROGRAM_CACHE = {}


def _get_program():
    if "nc" not in _PROGRAM_CACHE:
        _PROGRAM_CACHE["nc"] = _build_program()
    return _PROGRAM_CACHE["nc"]


def _host_prep(sinu_pos, proj):
    bf = ml_dtypes.bfloat16
    sinu = np.asarray(sinu_pos, np.float32)[0]          # [L, D]
    proj = np.asarray(proj, np.float32)                 # [M, D]
    half = D // 2
    sin_i = np.repeat(sinu[:, :half], 2, axis=-1)       # [L, D]
    cos_i = np.repeat(sinu[:, half:], 2, axis=-1)
    sinalt = sin_i.copy()
    sinalt[:, 0::2] *= -1.0
    cos2 = np.ascontiguousarray(np.concatenate([cos_i, cos_i], axis=1)).astype(bf)
    sin2 = np.ascontiguousarray(np.concatenate([sinalt, sinalt], axis=1)).astype(bf)
    projt = np.ascontiguousarray(proj.T / np.sqrt(np.float32(M))).astype(bf)
    maskat = np.triu(np.ones((C, C), np.float32))
    return cos2, sin2, projt, maskat


def build_in_maps(q, k, v, sinu_pos, proj):
    bf = ml_dtypes.bfloat16
    q = np.asarray(q, np.float32)
    k = np.asarray(k, np.float32)
    v = np.asarray(v, np.float32)
    cos2, sin2, projt, maskat = _host_prep(sinu_pos, proj)
    ones_col = np.ones((L, 1), np.float32)
    pairs = [(b, h) for b in range(B) for h in range(H)]
    in_maps = []
    for core in range(NCORES):
        im = {"cos2": cos2, "sin2": sin2, "projt": projt, "maskat": maskat}
        for p in range(PAIRS_PER_CORE):
            b, h = pairs[core * PAIRS_PER_CORE + p]
            im[f"q{p}"] = np.ascontiguousarray(q[b, :, h, :]).astype(bf)
            im[f"k{p}"] = np.ascontiguousarray(k[b, :, h, :]).astype(bf)
            im[f"v{p}"] = np.ascontiguousarray(
                np.concatenate([v[b, :, h, :], ones_col], axis=1)
            ).astype(bf)
        in_maps.append(im)
    return in_maps


def kernel(q, k, v, sinu_pos, proj):
    nc = _get_program()
    in_maps = build_in_maps(q, k, v, sinu_pos, proj)
    res = run_bass_kernel_spmd(nc, in_maps, core_ids=list(range(NCORES)))

    pairs = [(b, h) for b in range(B) for h in range(H)]
    out = np.empty((B, L, H, D), np.float32)
    for core in range(NCORES):
        for p in range(PAIRS_PER_CORE):
            b, h = pairs[core * PAIRS_PER_CORE + p]
            out[b, :, h, :] = res.results[core][f"o{p}"]
    return out


# revision 13
# speedup vs baseline: 2.1853x; 1.0286x over previous
"""FAVOR+ (Performer) causal linear attention with rotary embeddings on 8 TRN2 cores.

Reference computation (B=2, L=4096, H=8, D=64, M=256):
  q,k <- GPT-J rotary(q, k, sinu_pos)
  qp = relu(rot_q @ projT / sqrt(M)) + EPS   [B,L,H,M]
  kp = relu(rot_k @ projT / sqrt(M)) + EPS
  causal scan over L: KV_l = sum_{j<=l} kp_j (x) [v_j, 1];  out_l = (qp_l @ KV_l)[:D] / (qp_l @ KV_l)[D]

Sharding: 16 (b,h) pairs, 2 per core (pure data parallel, no collectives).
Per core: chunked scan with C=128 chunks; the two pairs are interleaved
chunk-by-chunk so their independent dependency chains fill each other's
cross-engine stalls. The KV state [M, D+1] lives in PSUM (one bank per
pair, both m-halves packed) and accumulates across chunks via matmul
accumulation (has_written bits).

Numerics: all matmul operands are bf16, accumulation fp32 in PSUM; the
final num/den division is fp32. Measured end-to-end relative error vs the
fp32 reference ~2e-3 (dominated by bf16 rounding of matmul operands).

Hardware notes baked in here:
 - fp32 matmuls on TRN2 are emulated as 2 bf16 passes (2x instructions,
   2x weight loads) -> use bf16 operands.
 - Matmuls on disjoint PE row groups execute CONCURRENTLY; two such
   matmuls draining into the same PSUM bank crash the device. q-features
   (rows 0:63) and k-features (rows 64:127) therefore write separate banks.
 - This walrus build supports ONE sync-wait slot per instruction;
   _legalize_sync_waits splits multi-wait instructions.
"""

import sys
import os

for _p in ("/opt/trn_rl_repo", "/root/.axon_site/_ro/trn_rl_repo"):
    if os.path.isdir(_p) and _p not in sys.path:
        sys.path.insert(0, _p)

import numpy as np
import ml_dtypes
import concourse.bass as bass
import concourse.mybir as mybir
import concourse.tile as tile
from concourse.bass_utils import run_bass_kernel_spmd
from concourse.masks import make_identity

B, L, H, D, M = 2, 4096, 8, 64, 256
EPS = 1e-3
C = 128                 # chunk length
NCH = L // C            # 32 chunks
NCORES = 8
PAIRS_PER_CORE = (B * H) // NCORES  # 2
F32 = mybir.dt.float32
BF16 = mybir.dt.bfloat16

# kv PSUM packing: m0 at cols [0:65], m1 at cols [68:133] (16B-aligned)
KV1 = 68
KVW = 136


def _legalize_sync_waits(nc):
    """Split multi-wait instructions into preceding single-wait
    EventSemaphore ops on the same engine (same-engine execution is
    in-order, so sequential waits == AND of waits)."""
    for f in nc.m.functions:
        for b in f.blocks:
            insts = b.instructions
            new = []
            dirty = False
            for ins in insts:
                si = ins.sync_info
                if si is not None and si.on_wait is not None and len(si.on_wait) > 1:
                    waits = list(si.on_wait)
                    for j, wt in enumerate(waits[:-1]):
                        es = mybir.InstEventSemaphore(
                            name=f"{ins.name}_xw{j}",
                            engine=ins.engine,
                            ins=[],
                            outs=[],
                            sync_info=mybir.SyncInfo(on_wait=[wt], on_update=[]),
                        )
                        new.append(es)
                    ins.sync_info = mybir.SyncInfo(
                        on_wait=[waits[-1]], on_update=list(si.on_update or [])
                    )
                    dirty = True
                if si is not None and si.on_update is not None and len(si.on_update) > 1:
                    raise AssertionError(
                        f"multi-update on {ins.name} ({ins.opcode}) unsupported"
                    )
                new.append(ins)
            if dirty:
                b.instructions = new


def _build_program(legalize=True):
    nc = bass.Bass()

    qk_in = []
    outs = []
    for p in range(PAIRS_PER_CORE):
        qd = nc.dram_tensor(f"q{p}", [L, D], BF16, kind="ExternalInput")
        kd = nc.dram_tensor(f"k{p}", [L, D], BF16, kind="ExternalInput")
        vd = nc.dram_tensor(f"v{p}", [L, D + 1], BF16, kind="ExternalInput")
        qk_in.append((qd, kd, vd))
        outs.append(nc.dram_tensor(f"o{p}", [L, D], F32, kind="ExternalOutput"))
    cos2_d = nc.dram_tensor("cos2", [L, 2 * D], BF16, kind="ExternalInput")
    sin2_d = nc.dram_tensor("sin2", [L, 2 * D], BF16, kind="ExternalInput")
    projt_d = nc.dram_tensor("projt", [D, M], BF16, kind="ExternalInput")
    mask_d = nc.dram_tensor("maskat", [C, C], F32, kind="ExternalInput")

    with tile.TileContext(nc) as tc:
        with (
            tc.tile_pool(name="consts", bufs=1) as consts,
            tc.tile_pool(name="stream", bufs=4) as stream,
            tc.tile_pool(name="feat", bufs=3) as feat,
            tc.tile_pool(name="kvp", bufs=2) as kvpool,
            tc.tile_pool(name="psA", bufs=1, space="PSUM") as psA,
            tc.tile_pool(name="psT", bufs=2, space="PSUM") as psT,
            tc.tile_pool(name="psO", bufs=2, space="PSUM") as psO,
            tc.tile_pool(name="pskv", bufs=1, space="PSUM") as pskv,
        ):
            # ---- constants ----
            cos_sb = consts.tile([128, NCH, 2 * D], BF16)
            sin_sb = consts.tile([128, NCH, 2 * D], BF16)
            nc.sync.dma_start(cos_sb[:], cos2_d.rearrange("(c p) j -> p c j", p=128))
            nc.sync.dma_start(sin_sb[:], sin2_d.rearrange("(c p) j -> p c j", p=128))
            projt2 = consts.tile([128, M], BF16)
            nc.sync.dma_start(projt2[0:D, :], projt_d[:])
            nc.sync.dma_start(projt2[D : 2 * D, :], projt_d[:])
            maskat = consts.tile([C, C], F32)
            nc.sync.dma_start(maskat[:], mask_d[:])
            ident = consts.tile([128, 128], BF16)
            make_identity(nc, ident[:])

            kv_ps = [
                pskv.tile([128, KVW], F32, name=f"kvps{p}", tag=f"kv{p}")
                for p in range(PAIRS_PER_CORE)
            ]

            for ci in range(NCH):
                for p in range(PAIRS_PER_CORE):
                    qd, kd, vd = qk_in[p]
                    od = outs[p]
                    kv = kv_ps[p]
                    lo = ci * C

                    # -------- load (bf16) --------
                    xqk = stream.tile([128, 128], BF16, tag="xqk")
                    nc.sync.dma_start(xqk[:, 0:D], qd[lo : lo + C, :])
                    nc.sync.dma_start(xqk[:, D : 2 * D], kd[lo : lo + C, :])
                    v_aug = stream.tile([128, D + 1], BF16, tag="vaug")
                    nc.sync.dma_start(v_aug[:], vd[lo : lo + C, :])

                    # -------- rotary: rot = x*cos2 + swap(x)*sin2alt --------
                    cslice = cos_sb[:, ci, :]
                    sslice = sin_sb[:, ci, :]
                    x_sw = xqk.rearrange("p (t two) -> p t two", two=2)[:, :, ::-1]
                    t2 = stream.tile([128, 128], BF16, tag="t2")
                    nc.gpsimd.tensor_tensor(
                        t2[:].rearrange("p (t two) -> p t two", two=2),
                        x_sw,
                        sslice.rearrange("p (t two) -> p t two", two=2),
                        mybir.AluOpType.mult,
                    )
                    t1 = stream.tile([128, 128], BF16, tag="t1")
                    nc.vector.tensor_tensor(t1[:], xqk[:], cslice, mybir.AluOpType.mult)
                    rot = stream.tile([128, 128], BF16, tag="rot")
                    nc.gpsimd.tensor_tensor(rot[:], t1[:], t2[:], mybir.AluOpType.add)

                    # -------- PE transpose: rotT rows 0:63 = qT, 64:127 = kT ----
                    pt = psT.tile([128, 128], BF16, tag="pt")
                    nc.tensor.transpose(pt[:], rot[:], ident[:])
                    rotT = feat.tile([128, 128], BF16, tag="rotT")
                    nc.scalar.copy(rotT[:], pt[:])

                    # -------- features: q rows 0:63 / k rows 64:127 ----------
                    # concurrent row groups MUST drain to different banks
                    ps_fq = psA.tile([128, 256], F32, tag="pfq")
                    ps_fk = psA.tile([128, 512], F32, tag="pfk")
                    for m in range(2):
                        nc.tensor.matmul(
                            ps_fq[:, m * 128 : (m + 1) * 128],
                            projt2[0:D, m * 128 : (m + 1) * 128],
                            rotT[0:D, :],
                            start=True, stop=True,
                        )
                        nc.tensor.matmul(
                            ps_fk[:, m * 128 : (m + 1) * 128],
                            projt2[D : 2 * D, m * 128 : (m + 1) * 128],
                            rotT[D : 2 * D, :],
                            start=True, stop=True,
                        )
                    if ci < NCH - 1:
                        # kp[C, M] (lhsT of the KV update), k row-group
                        nc.tensor.matmul(
                            ps_fk[:, 256:512],
                            rotT[D : 2 * D, :],
                            projt2[D : 2 * D, :],
                            start=True, stop=True,
                        )

                    fsb = feat.tile([128, 512], BF16, tag="fsb")
                    nc.vector.tensor_scalar(
                        fsb[:, 0:256], ps_fq[:], 0.0, EPS,
                        mybir.AluOpType.max, mybir.AluOpType.add,
                    )
                    nc.vector.tensor_scalar(
                        fsb[:, 256:512], ps_fk[:, 0:256], 0.0, EPS,
                        mybir.AluOpType.max, mybir.AluOpType.add,
                    )
                    qpT = [fsb[:, 0:128], fsb[:, 128:256]]
                    kpT = [fsb[:, 256:384], fsb[:, 384:512]]
                    if ci < NCH - 1:
                        kp_sb = feat.tile([C, M], BF16, tag="kpsb")
                        nc.vector.tensor_scalar(
                            kp_sb[:], ps_fk[:, 256:512], 0.0, EPS,
                            mybir.AluOpType.max, mybir.AluOpType.add,
                        )

                    # -------- AT = kp qp^T, causal mask --------
                    # po bank: cols 0:65 = num/den, cols 128:256 = AT
                    po = psO.tile([C, 256], F32, tag="po")
                    ps_a = po[:, 128:256]
                    nc.tensor.matmul(ps_a, kpT[0], qpT[0], start=True, stop=False)
                    nc.tensor.matmul(ps_a, kpT[1], qpT[1], start=False, stop=True)
                    at_sb = feat.tile([C, C], BF16, tag="atsb")
                    nc.vector.tensor_tensor(
                        at_sb[:], ps_a, maskat[:], mybir.AluOpType.mult
                    )

                    # -------- snapshot KV (chunks < ci) --------
                    if ci > 0:
                        kv_sb = kvpool.tile([128, KVW], BF16, tag="kvsb")
                        nc.scalar.copy(
                            kv_sb[:, 0 : KV1 + D + 1], kv[:, 0 : KV1 + D + 1]
                        )

                    # -------- num/den --------
                    ps_o = po[:, 0 : D + 1]
                    if ci > 0:
                        nc.tensor.matmul(
                            ps_o, qpT[0], kv_sb[:, 0 : D + 1], start=True, stop=False
                        )
                        nc.tensor.matmul(
                            ps_o, qpT[1], kv_sb[:, KV1 : KV1 + D + 1],
                            start=False, stop=False,
                        )
                        nc.tensor.matmul(
                            ps_o, at_sb[:], v_aug[:], start=False, stop=True
                        )
                    else:
                        nc.tensor.matmul(
                            ps_o, at_sb[:], v_aug[:], start=True, stop=True
                        )

                    # -------- KV += kp^T v_aug (PSUM accumulate) --------
                    if ci < NCH - 1:
                        for m in range(2):
                            nc.tensor.matmul(
                                kv[:, m * KV1 : m * KV1 + D + 1],
                                kp_sb[:, m * 128 : (m + 1) * 128],
                                v_aug[:],
                                start=(ci == 0 and m == 0),
                                stop=True,
                                skip_group_check=True,
                            )

                    # -------- out = num * (1/den) --------
                    rec = feat.tile([C, 1], F32, tag="rec")
                    nc.vector.reciprocal(rec[:], po[:, D : D + 1])
                    osb = feat.tile([C, D], F32, tag="osb")
                    nc.scalar.activation(
                        osb[:], po[:, 0:D],
                        mybir.ActivationFunctionType.Copy,
                        bias=0.0, scale=rec[:],
                    )
                    nc.sync.dma_start(od[lo : lo + C, :], osb[:])

    if legalize:
        _legalize_sync_waits(nc)
    return nc


_PROGRAM_CACHE = {}


def _get_program():
    if "nc" not in _PROGRAM_CACHE:
        _PROGRAM_CACHE["nc"] = _build_program()
    return _PROGRAM_CACHE["nc"]


def _host_prep(sinu_pos, proj):
    bf = ml_dtypes.bfloat16
    sinu = np.asarray(sinu_pos, np.float32)[0]          # [L, D]
    proj = np.asarray(proj, np.float32)                 # [M, D]
    half = D // 2
    sin_i = np.repeat(sinu[:, :half], 2, axis=-1)       # [L, D]
    cos_i = np.repeat(sinu[:, half:], 2, axis=-1)
    sinalt = sin_i.copy()
    sinalt[:, 0::2] *= -1.0
    cos2 = np.ascontiguousarray(np.concatenate([cos_i, cos_i], axis=1)).astype(bf)
    sin2 = np.ascontiguousarray(np.concatenate([sinalt, sinalt], axis=1)).astype(bf)
    projt = np.ascontiguousarray(proj.T / np.sqrt(np.float32(M))).astype(bf)
    maskat = np.triu(np.ones((C, C), np.float32))
    return cos2, sin2, projt, maskat


def build_in_maps(q, k, v, sinu_pos, proj):
    bf = ml_dtypes.bfloat16
    q = np.asarray(q, np.float32)
    k = np.asarray(k, np.float32)
    v = np.asarray(v, np.float32)
    cos2, sin2, projt, maskat = _host_prep(sinu_pos, proj)
    ones_col = np.ones((L, 1), np.float32)
    pairs = [(b, h) for b in range(B) for h in range(H)]
    in_maps = []
    for core in range(NCORES):
        im = {"cos2": cos2, "sin2": sin2, "projt": projt, "maskat": maskat}
        for p in range(PAIRS_PER_CORE):
            b, h = pairs[core * PAIRS_PER_CORE + p]
            im[f"q{p}"] = np.ascontiguousarray(q[b, :, h, :]).astype(bf)
            im[f"k{p}"] = np.ascontiguousarray(k[b, :, h, :]).astype(bf)
            im[f"v{p}"] = np.ascontiguousarray(
                np.concatenate([v[b, :, h, :], ones_col], axis=1)
            ).astype(bf)
        in_maps.append(im)
    return in_maps


def kernel(q, k, v, sinu_pos, proj):
    nc = _get_program()
    in_maps = build_in_maps(q, k, v, sinu_pos, proj)
    res = run_bass_kernel_spmd(nc, in_maps, core_ids=list(range(NCORES)))

    pairs = [(b, h) for b in range(B) for h in range(H)]
    out = np.empty((B, L, H, D), np.float32)
    for core in range(NCORES):
        for p in range(PAIRS_PER_CORE):
            b, h = pairs[core * PAIRS_PER_CORE + p]
            out[b, :, h, :] = res.results[core][f"o{p}"]
    return out
